# revision 32
# baseline (speedup 1.0000x reference)
import sys

if "/opt/trn_rl_repo" not in sys.path:
    sys.path.insert(0, "/opt/trn_rl_repo")

from collections import deque

import numpy as np

import concourse.bacc as bacc
import concourse.tile as tile
from concourse import bass_utils, mybir
from concourse.bass import ts
from concourse.masks import make_identity

F32 = mybir.dt.float32
BF16 = mybir.dt.bfloat16
EXP = mybir.ActivationFunctionType.Exp


# nn_MultiHeadedAttention: B=2, S=2048, D=1024, H=16, DH=64.
# 16 heads over 8 cores (2 heads/core = 128 features). QKV column-parallel,
# out-projection row-parallel, host sums the 8 partial outputs.
#
# Schedule: the attention j-loop (scores pair -> exp -> o-accum pair) is the
# backbone; projection / transpose / out-projection work is held in a queue
# of small units and pumped between attention matmuls so the PE never idles
# while the ACT engine runs exp, and the HAM clock never re-throttles.
B, S, D, H = 2, 2048, 1024, 16
DH = D // H
NC = 8
T = B * S                  # 4096 tokens
NCHUNK = T // 512          # 8 token chunks of 512
KCH = D // 128             # 8 contraction chunks
NJ = S // 128              # 16 key tiles per batch
QC = S // 512              # 4 query chunks per batch

_CACHE = {}


def _build():
    if "nc" in _CACHE:
        return _CACHE["nc"]

    nc = bacc.Bacc("TRN2", target_bir_lowering=False, debug=False,
                   enable_asserts=True, num_devices=NC)

    xT = nc.dram_tensor("xT", [D, T], BF16, kind="ExternalInput").ap()
    wq = nc.dram_tensor("wq", [D, 128], BF16, kind="ExternalInput").ap()
    wk = nc.dram_tensor("wk", [D, 128], BF16, kind="ExternalInput").ap()
    wv = nc.dram_tensor("wv", [D, 128], BF16, kind="ExternalInput").ap()
    wo = nc.dram_tensor("wo", [128, D], BF16, kind="ExternalInput").ap()
    bq = nc.dram_tensor("bq", [128, 1], F32, kind="ExternalInput").ap()
    bk = nc.dram_tensor("bk", [128, 1], F32, kind="ExternalInput").ap()
    outT = nc.dram_tensor("outT", [D, T], F32, kind="ExternalOutput").ap()

    with tile.TileContext(nc) as tc:
        with (
            tc.tile_pool(name="wpool", bufs=1) as wpool,
            tc.tile_pool(name="qk", bufs=1) as qk_pool,
            tc.tile_pool(name="vtm", bufs=1) as vtm_pool,
            tc.tile_pool(name="on", bufs=1) as on_pool,
            tc.tile_pool(name="xin", bufs=1) as xin_pool,
            tc.tile_pool(name="vst", bufs=2) as vst_pool,
            tc.tile_pool(name="epool", bufs=8) as epool,
            tc.tile_pool(name="npool", bufs=2) as npool,
            tc.tile_pool(name="ostage", bufs=3) as ostage_pool,
            # PSUM (8 banks): psA 2x[128,1024] (scores) = 4, psO [65,1024]
            # (o accum, single-buffered) = 2, psP 2x[128,512] (projections,
            # transposes, out-projection) = 2.
            tc.tile_pool(name="psA", bufs=2, space="PSUM") as psA,
            tc.tile_pool(name="psO", bufs=1, space="PSUM") as psO,
            tc.tile_pool(name="psP", bufs=2, space="PSUM") as psP,
        ):
            # ---- persistent weights / constants ----
            wq_sb = wpool.tile([128, D], BF16)
            wk_sb = wpool.tile([128, D], BF16)
            wv_sb = wpool.tile([128, D], BF16)
            wo_sb = wpool.tile([128, D], BF16)
            bq_sb = wpool.tile([128, 1], F32)
            bk_sb = wpool.tile([128, 1], F32)
            ident = wpool.tile([128, 128], F32)
            ones1 = wpool.tile([1, 64], F32)
            nc.vector.memset(ones1[:], 1.0)
            xp = []
            for n in range(NCHUNK):
                xp.append(xin_pool.tile([128, KCH * 512], BF16,
                                        name=f"xp{n}"))
            for k in range(KCH):
                nc.sync.dma_start(xp[0][:, ts(k, 512)],
                                  xT[128 * k:128 * (k + 1), ts(0, 512)])
            nc.sync.dma_start(wq_sb[:], wq.rearrange("(k p) f -> p k f", p=128))
            nc.sync.dma_start(wk_sb[:], wk.rearrange("(k p) f -> p k f", p=128))
            nc.sync.dma_start(wv_sb[:], wv.rearrange("(k p) f -> p k f", p=128))
            nc.sync.dma_start(wo_sb[:], wo[:])
            nc.sync.dma_start(bq_sb[:], bq[:])
            nc.sync.dma_start(bk_sb[:], bk[:])
            make_identity(nc, ident[:])

            # Warm the ACT exp table before phase 1.
            dummy = wpool.tile([1, 2], F32)
            nc.vector.memset(dummy[:], 0.0)
            nc.scalar.activation(dummy[:], dummy[:], EXP)

            # remaining x chunks ([128, 8*512] bf16 each, k-major), one DMA
            # per k-slice so transfers spread across queues
            for n in range(1, NCHUNK):
                for k in range(KCH):
                    nc.sync.dma_start(
                        xp[n][:, ts(k, 512)],
                        xT[128 * k:128 * (k + 1), ts(n, 512)])

            # persistent activations
            qn = [qk_pool.tile([128, 512], BF16, name=f"qn{n}")
                  for n in range(NCHUNK)]
            kn = [qk_pool.tile([128, 512], BF16, name=f"kn{n}")
                  for n in range(NCHUNK)]
            on = [on_pool.tile([128, 512], BF16, name=f"on{n}")
                  for n in range(NCHUNK)]
            # v stationaries [128 keys, 128]: col 0 = ones (softmax sums ->
            # psum partition 0), cols 64-127 = v^T (o -> psum partitions
            # 64-127); zeros between keep partition bases 32-aligned.
            v_tm = {}
            for hh in range(2):
                for J in range(2 * NJ):
                    v_tm[(hh, J)] = vtm_pool.tile(
                        [128, 128], BF16, name=f"vtm{hh}_{J}")


            # ---- bridge work: units of (est_ns, closure, label) ----
            units = deque()
            qstate = {"ns": 0, "debt": 0}
            done = set()

            def push(us):
                units.extend(us)
                qstate["ns"] += sum(u[0] for u in us)

            def _pop1():
                est, fn, label = units.popleft()
                fn()
                qstate["ns"] -= est
                if label is not None:
                    done.add(label)
                return est

            def pump(budget_ns):
                # debt carries over-pops so the average drain rate matches
                # the requested budget even though units are coarse
                budget_ns -= qstate["debt"]
                spent = 0
                while units and budget_ns > spent:
                    spent += _pop1()
                qstate["debt"] = max(0, spent - max(budget_ns, 0))

            def ensure(label):
                # hard deadline: emit queued units until `label` has been
                # produced (attention reads must follow their producers in
                # PE program order)
                spent = 0
                while label not in done and units:
                    spent += _pop1()
                qstate["debt"] += spent

            def proj_units(n, w_sb, b_sb, dst, kind):
                """One projection (q/k/v) of chunk n: 8 MMs into a psP tile,
                then bias-add (q/k) or copy (v) drain to SBUF."""
                state = {}

                def first_half():
                    p = psP.tile([128, 512], F32, tag="P",
                                 name=f"pj{kind}{n}")
                    state["p"] = p
                    for k in range(4):
                        nc.tensor.matmul(p[:], w_sb[:, ts(k, 128)],
                                         xp[n][:, ts(k, 512)],
                                         start=(k == 0), stop=False)

                def second_half():
                    p = state["p"]
                    for k in range(4, KCH):
                        nc.tensor.matmul(p[:], w_sb[:, ts(k, 128)],
                                         xp[n][:, ts(k, 512)],
                                         start=False, stop=(k == KCH - 1))
                    if kind == "v":
                        vs = vst_pool.tile([128, 512], F32, tag="vst",
                                           name=f"vst{n}")
                        state["vst"] = vs
                        nc.vector.tensor_copy(vs[:], p[:])
                    else:
                        nc.vector.tensor_scalar_add(dst[:], p[:], b_sb[:])

                return [(900, first_half, None),
                        (900, second_half, (kind, n))], state

            def transpose_units(n, vstate):
                """4 full-width transposes of chunk n's v + v_tm copies."""
                out = []
                for jj in range(4):
                    def mk(jj=jj):
                        def f():
                            vs = vstate["vst"]
                            t_ps = psP.tile([128, 512], F32, tag="P",
                                            name=f"tp{n}_{jj}")
                            nc.tensor.transpose(t_ps[:, 0:128],
                                                vs[:, ts(jj, 128)],
                                                ident[:])
                            for hh in range(2):
                                vt = v_tm[(hh, 4 * n + jj)]
                                nc.vector.memset(vt[:, 0:1], 1.0)
                                nc.vector.memset(vt[:, 1:64], 0.0)
                                nc.vector.tensor_copy(
                                    vt[:, 64:128], t_ps[:, ts(hh, 64)])
                        return f
                    out.append((350, mk(), ("t", 4 * n + jj)))
                return out

            def ph1_units(n):
                """Units for phase-1 chunk n, split into (k+v+transposes)
                needed by any same-batch attention chunk, and the
                q-projection only needed when chunk n's queries run."""
                ku, _ = proj_units(n, wk_sb, bk_sb, kn[n], "k")
                qu, _ = proj_units(n, wq_sb, bq_sb, qn[n], "q")
                vu, vstate = proj_units(n, wv_sb, None, None, "v")
                return ku + vu + transpose_units(n, vstate), qu

            def outproj_units(c):
                out = []
                for m in range(KCH):
                    def mk(m=m):
                        def f():
                            op = psP.tile([128, 512], F32, tag="P",
                                          name=f"op{c}_{m}")
                            nc.tensor.matmul(op[:], wo_sb[:, ts(m, 128)],
                                             on[c][:], start=True, stop=True)
                            ost = ostage_pool.tile([128, 512], F32, tag="ost",
                                                   name=f"ost{c}_{m}")
                            nc.vector.tensor_copy(ost[:], op[:])
                            nc.sync.dma_start(outT[ts(m, 128), ts(c, 512)],
                                              ost[:])
                        return f
                    out.append((250, mk(), None))
                return out

            # ---- attention ----
            def emit_attn(c):
                b, qc = divmod(c, QC)
                n = c
                ensure(("q", n))
                o_ps = psO.tile([128, 1024], F32, tag="O", name=f"ops{n}")
                e_hist = []
                for j in range(NJ):
                    ensure(("k", b * QC + j // 4))
                    if j >= 3:
                        ensure(("t", b * QC * 4 + j - 3))
                    s_ps = psA.tile([128, 1024], F32, tag="A",
                                    name=f"sps{n}_{j}")
                    for hh in range(2):
                        hs = slice(hh * 64, (hh + 1) * 64)
                        nc.tensor.matmul(
                            s_ps[:, ts(hh, 512)],
                            kn[b * QC + j // 4][hs, ts(j % 4, 128)],
                            qn[n][hs, :], start=True, stop=True)
                    e_sb = epool.tile([128, 1024], BF16, tag="e",
                                      name=f"e{n}_{j}")
                    nc.scalar.activation(e_sb[:], s_ps[:], EXP)
                    rem_j = (2 * QC - c) * NJ - j
                    adaptive = qstate["ns"] // max(rem_j, 1)
                    floor = 800 if c == 0 else 380
                    pump(max(adaptive, floor))
                    # o-accum lags by 3 j-steps: the first o-matmul of a
                    # chunk lands after the previous chunk's norm reads
                    # have freed psO, so the PE never waits on that WAR
                    if j >= 3:
                        for hh in range(2):
                            nc.tensor.matmul(
                                o_ps[0:128, ts(hh, 512)],
                                v_tm[(hh, b * NJ + j - 3)][:],
                                e_hist[j - 3][:, ts(hh, 512)],
                                start=(j - 3 == 0), stop=False)
                    e_hist.append(e_sb)
                for jj in (NJ - 3, NJ - 2, NJ - 1):
                    ensure(("t", b * NJ + jj))
                    for hh in range(2):
                        nc.tensor.matmul(
                            o_ps[0:128, ts(hh, 512)],
                            v_tm[(hh, b * NJ + jj)][:],
                            e_hist[jj][:, ts(hh, 512)],
                            start=False, stop=(jj == NJ - 1))
                # sums live at psum partition 0: reciprocal reads PSUM
                # directly; broadcast via tiny PE matmuls against ones1
                r_sb = npool.tile([1, 1024], F32, tag="r", name=f"r{n}")
                nc.vector.reciprocal_approx_fast(r_sb[:], o_ps[0:1, :])
                oc = npool.tile([64, 1024], F32, tag="oc", name=f"oc{n}")
                nc.vector.tensor_copy(oc[:], o_ps[64:128, :])
                for hh in range(2):
                    hs = slice(hh * 64, (hh + 1) * 64)
                    rb = npool.tile([64, 512], F32, tag=f"rb{hh}",
                                    name=f"rb{n}_{hh}")
                    nc.gpsimd.partition_broadcast(
                        rb[:], r_sb[0:1, ts(hh, 512)])
                    nc.vector.tensor_tensor(
                        out=on[n][hs, :], in0=oc[0:64, ts(hh, 512)],
                        in1=rb[:], op=mybir.AluOpType.mult)

            # ---- schedule ----
            # head: chunks 0 and 1 projected directly; the rest bridges.
            # Bridge order: k/v/transposes of chunks 2,3 first (batch-0
            # attention needs them early), their q-projections after, then
            # batch-1 chunks in full.
            for n in (0, 1):
                kvt, qu = ph1_units(n)
                for u in kvt + qu:
                    u[1]()
                    if u[2] is not None:
                        done.add(u[2])
            kvt2, qu2 = ph1_units(2)
            kvt3, qu3 = ph1_units(3)
            push(kvt2)
            push(kvt3)
            push(qu2)
            push(qu3)
            for n in range(4, NCHUNK):
                kvt, qu = ph1_units(n)
                push(kvt)
                push(qu)
            # out-projection of chunk c-2 is prepended at chunk c's start:
            # it is guaranteed normalized and fills the chunk-boundary
            # window (o-psum WAR + norm chain) so the PE never goes cold.
            last = 2 * QC - 1
            for c in range(2 * QC):
                pre = []
                if c >= 2:
                    pre += outproj_units(c - 2)
                if c == last:
                    pre += outproj_units(last - 1)[:4]
                for u in reversed(pre):
                    units.appendleft(u)
                    qstate["ns"] += u[0]
                emit_attn(c)
            push(outproj_units(last - 1)[4:])
            push(outproj_units(last))
            pump(10 ** 9)   # drain whatever is left

    nc.compile()
    _CACHE["nc"] = nc
    return nc


def _prep_in_maps(x, Wq, bq, Wk, bk, Wv, Wo):
    import ml_dtypes
    bf16 = ml_dtypes.bfloat16
    xT = np.ascontiguousarray(x.reshape(T, D).T).astype(bf16)
    scale = np.float32(1.0 / np.sqrt(DH))
    in_maps = []
    for c in range(NC):
        sl = slice(128 * c, 128 * (c + 1))
        in_maps.append({
            "xT": xT,
            "wq": np.ascontiguousarray((scale * Wq[sl, :]).T).astype(bf16),
            "wk": np.ascontiguousarray(Wk[sl, :].T).astype(bf16),
            "wv": np.ascontiguousarray(Wv[sl, :].T).astype(bf16),
            "wo": np.ascontiguousarray(Wo[:, sl].T).astype(bf16),
            "bq": np.ascontiguousarray((scale * bq[sl])[:, None]),
            "bk": np.ascontiguousarray(bk[sl][:, None]),
        })
    return in_maps


def kernel(x, Wq, bq, Wk, bk, Wv, bv, Wo, bo):
    x = np.asarray(x, np.float32)
    Wq, bq = np.asarray(Wq, np.float32), np.asarray(bq, np.float32)
    Wk, bk = np.asarray(Wk, np.float32), np.asarray(bk, np.float32)
    Wv, bv = np.asarray(Wv, np.float32), np.asarray(bv, np.float32)
    Wo, bo = np.asarray(Wo, np.float32), np.asarray(bo, np.float32)

    nc = _build()
    in_maps = _prep_in_maps(x, Wq, bq, Wk, bk, Wv, Wo)
    res = bass_utils.run_bass_kernel_spmd(nc, in_maps, core_ids=list(range(NC)))

    acc = np.zeros((D, T), np.float64)
    for c in range(NC):
        acc += res.results[c]["outT"]
    # v-bias folds through softmax (rows sum to 1): + bv @ Wo.T; plus bo.
    const = bo.astype(np.float64) + bv.astype(np.float64) @ Wo.T.astype(np.float64)
    out = acc.T + const[None, :]
    return out.astype(np.float32).reshape(B, S, D)


# revision 34
# speedup vs baseline: 1.0215x; 1.0215x over previous
import sys

if "/opt/trn_rl_repo" not in sys.path:
    sys.path.insert(0, "/opt/trn_rl_repo")

from collections import deque

import numpy as np

import concourse.bacc as bacc
import concourse.tile as tile
from concourse import bass_utils, mybir
from concourse.bass import ts
from concourse.masks import make_identity

F32 = mybir.dt.float32
BF16 = mybir.dt.bfloat16
EXP = mybir.ActivationFunctionType.Exp


# nn_MultiHeadedAttention: B=2, S=2048, D=1024, H=16, DH=64.
# 16 heads over 8 cores (2 heads/core = 128 features). QKV column-parallel,
# out-projection row-parallel, host sums the 8 partial outputs.
#
# Schedule: the attention j-loop (scores pair -> exp -> o-accum pair) is the
# backbone; projection / transpose / out-projection work is held in a queue
# of small units and pumped between attention matmuls so the PE never idles
# while the ACT engine runs exp, and the HAM clock never re-throttles.
B, S, D, H = 2, 2048, 1024, 16
DH = D // H
NC = 8
T = B * S                  # 4096 tokens
NCHUNK = T // 512          # 8 token chunks of 512
KCH = D // 128             # 8 contraction chunks
NJ = S // 128              # 16 key tiles per batch
QC = S // 512              # 4 query chunks per batch

_CACHE = {}


def _build():
    if "nc" in _CACHE:
        return _CACHE["nc"]

    nc = bacc.Bacc("TRN2", target_bir_lowering=False, debug=False,
                   enable_asserts=True, num_devices=NC)

    xT = nc.dram_tensor("xT", [D, T], BF16, kind="ExternalInput").ap()
    wq = nc.dram_tensor("wq", [D, 128], BF16, kind="ExternalInput").ap()
    wk = nc.dram_tensor("wk", [D, 128], BF16, kind="ExternalInput").ap()
    wv = nc.dram_tensor("wv", [D, 128], BF16, kind="ExternalInput").ap()
    wo = nc.dram_tensor("wo", [128, D], BF16, kind="ExternalInput").ap()
    bq = nc.dram_tensor("bq", [128, 1], F32, kind="ExternalInput").ap()
    bk = nc.dram_tensor("bk", [128, 1], F32, kind="ExternalInput").ap()
    outT = nc.dram_tensor("outT", [D, T], F32, kind="ExternalOutput").ap()

    with tile.TileContext(nc) as tc:
        with (
            tc.tile_pool(name="wpool", bufs=1) as wpool,
            tc.tile_pool(name="qk", bufs=1) as qk_pool,
            tc.tile_pool(name="vtm", bufs=1) as vtm_pool,
            tc.tile_pool(name="on", bufs=1) as on_pool,
            tc.tile_pool(name="xin", bufs=1) as xin_pool,
            tc.tile_pool(name="vst", bufs=2) as vst_pool,
            tc.tile_pool(name="epool", bufs=6) as epool,
            tc.tile_pool(name="npool", bufs=2) as npool,
            tc.tile_pool(name="ostage", bufs=3) as ostage_pool,
            # PSUM (8 banks): psA 2x[128,1024] (scores) = 4, psO [65,1024]
            # (o accum, single-buffered) = 2, psP 2x[128,512] (projections,
            # transposes, out-projection) = 2.
            tc.tile_pool(name="psA", bufs=2, space="PSUM") as psA,
            tc.tile_pool(name="psO", bufs=1, space="PSUM") as psO,
            tc.tile_pool(name="psP", bufs=2, space="PSUM") as psP,
        ):
            # ---- persistent weights / constants ----
            wq_sb = wpool.tile([128, D], BF16)
            wk_sb = wpool.tile([128, D], BF16)
            wv_sb = wpool.tile([128, D], BF16)
            wo_sb = wpool.tile([128, D], BF16)
            bq_sb = wpool.tile([128, 1], F32)
            bk_sb = wpool.tile([128, 1], F32)
            ident = wpool.tile([128, 128], F32)
            ones1 = wpool.tile([1, 64], F32)
            nc.vector.memset(ones1[:], 1.0)
            nc.sync.dma_start(wq_sb[:], wq.rearrange("(k p) f -> p k f", p=128))
            nc.sync.dma_start(wk_sb[:], wk.rearrange("(k p) f -> p k f", p=128))
            nc.sync.dma_start(wv_sb[:], wv.rearrange("(k p) f -> p k f", p=128))
            nc.sync.dma_start(wo_sb[:], wo[:])
            nc.sync.dma_start(bq_sb[:], bq[:])
            nc.sync.dma_start(bk_sb[:], bk[:])
            make_identity(nc, ident[:])

            # Warm the ACT exp table before phase 1.
            dummy = wpool.tile([1, 2], F32)
            nc.vector.memset(dummy[:], 0.0)
            nc.scalar.activation(dummy[:], dummy[:], EXP)

            # all x chunks loaded up front ([128, 8*512] bf16 each, k-major),
            # one DMA per k-slice so transfers spread across queues and the
            # first projection can start early
            xp = []
            for n in range(NCHUNK):
                t_ = xin_pool.tile([128, KCH * 512], BF16, name=f"xp{n}")
                for k in range(KCH):
                    nc.sync.dma_start(
                        t_[:, ts(k, 512)],
                        xT[128 * k:128 * (k + 1), ts(n, 512)])
                xp.append(t_)

            # persistent activations
            qn = [qk_pool.tile([128, 512], BF16, name=f"qn{n}")
                  for n in range(NCHUNK)]
            kn = [qk_pool.tile([128, 512], BF16, name=f"kn{n}")
                  for n in range(NCHUNK)]
            on = [on_pool.tile([128, 512], BF16, name=f"on{n}")
                  for n in range(NCHUNK)]
            # v stationaries [128 keys, 128]: col 0 = ones (softmax sums ->
            # psum partition 0), cols 64-127 = v^T (o -> psum partitions
            # 64-127); zeros between keep partition bases 32-aligned.
            v_tm = {}
            for hh in range(2):
                for J in range(2 * NJ):
                    v_tm[(hh, J)] = vtm_pool.tile(
                        [128, 128], BF16, name=f"vtm{hh}_{J}")
            for hh in range(2):
                for J in range(2 * NJ):
                    nc.vector.memset(v_tm[(hh, J)][:], 0.0)
                    nc.vector.memset(v_tm[(hh, J)][:, 0:1], 1.0)

            # ---- bridge work: units of (est_ns, closure, label) ----
            units = deque()
            qstate = {"ns": 0, "debt": 0}
            done = set()

            def push(us):
                units.extend(us)
                qstate["ns"] += sum(u[0] for u in us)

            def _pop1():
                est, fn, label = units.popleft()
                fn()
                qstate["ns"] -= est
                if label is not None:
                    done.add(label)
                return est

            def pump(budget_ns):
                # debt carries over-pops so the average drain rate matches
                # the requested budget even though units are coarse
                budget_ns -= qstate["debt"]
                spent = 0
                while units and budget_ns > spent:
                    spent += _pop1()
                qstate["debt"] = max(0, spent - max(budget_ns, 0))

            def ensure(label):
                # hard deadline: emit queued units until `label` has been
                # produced (attention reads must follow their producers in
                # PE program order)
                spent = 0
                while label not in done and units:
                    spent += _pop1()
                qstate["debt"] += spent

            def proj_units(n, w_sb, b_sb, dst, kind):
                """One projection (q/k/v) of chunk n: 8 MMs into a psP tile,
                then bias-add (q/k) or copy (v) drain to SBUF."""
                state = {}

                def first_half():
                    p = psP.tile([128, 512], F32, tag="P",
                                 name=f"pj{kind}{n}")
                    state["p"] = p
                    for k in range(4):
                        nc.tensor.matmul(p[:], w_sb[:, ts(k, 128)],
                                         xp[n][:, ts(k, 512)],
                                         start=(k == 0), stop=False)

                def second_half():
                    p = state["p"]
                    for k in range(4, KCH):
                        nc.tensor.matmul(p[:], w_sb[:, ts(k, 128)],
                                         xp[n][:, ts(k, 512)],
                                         start=False, stop=(k == KCH - 1))
                    if kind == "v":
                        vs = vst_pool.tile([128, 512], F32, tag="vst",
                                           name=f"vst{n}")
                        state["vst"] = vs
                        nc.vector.tensor_copy(vs[:], p[:])
                    else:
                        nc.vector.tensor_scalar_add(dst[:], p[:], b_sb[:])

                return [(900, first_half, None),
                        (900, second_half, (kind, n))], state

            def transpose_units(n, vstate):
                """4 full-width transposes of chunk n's v + v_tm copies."""
                out = []
                for jj in range(4):
                    def mk(jj=jj):
                        def f():
                            vs = vstate["vst"]
                            t_ps = psP.tile([128, 512], F32, tag="P",
                                            name=f"tp{n}_{jj}")
                            nc.tensor.transpose(t_ps[:, 0:128],
                                                vs[:, ts(jj, 128)],
                                                ident[:])
                            for hh in range(2):
                                vt = v_tm[(hh, 4 * n + jj)]
                                nc.vector.tensor_copy(
                                    vt[:, 64:128], t_ps[:, ts(hh, 64)])
                        return f
                    out.append((350, mk(), ("t", 4 * n + jj)))
                return out

            def ph1_units(n):
                """Units for phase-1 chunk n, split into (k+v+transposes)
                needed by any same-batch attention chunk, and the
                q-projection only needed when chunk n's queries run."""
                ku, _ = proj_units(n, wk_sb, bk_sb, kn[n], "k")
                qu, _ = proj_units(n, wq_sb, bq_sb, qn[n], "q")
                vu, vstate = proj_units(n, wv_sb, None, None, "v")
                return ku + vu + transpose_units(n, vstate), qu

            def outproj_units(c):
                out = []
                for m in range(KCH):
                    def mk(m=m):
                        def f():
                            op = psP.tile([128, 512], F32, tag="P",
                                          name=f"op{c}_{m}")
                            nc.tensor.matmul(op[:], wo_sb[:, ts(m, 128)],
                                             on[c][:], start=True, stop=True)
                            ost = ostage_pool.tile([128, 512], F32, tag="ost",
                                                   name=f"ost{c}_{m}")
                            nc.vector.tensor_copy(ost[:], op[:])
                            nc.sync.dma_start(outT[ts(m, 128), ts(c, 512)],
                                              ost[:])
                        return f
                    out.append((250, mk(), None))
                return out

            # ---- attention ----
            def emit_attn(c):
                b, qc = divmod(c, QC)
                n = c
                ensure(("q", n))
                o_ps = psO.tile([128, 1024], F32, tag="O", name=f"ops{n}")
                e_hist = []
                for j in range(NJ):
                    ensure(("k", b * QC + j // 4))
                    if j >= 2:
                        ensure(("t", b * QC * 4 + j - 2))
                    s_ps = psA.tile([128, 1024], F32, tag="A",
                                    name=f"sps{n}_{j}")
                    for hh in range(2):
                        hs = slice(hh * 64, (hh + 1) * 64)
                        nc.tensor.matmul(
                            s_ps[:, ts(hh, 512)],
                            kn[b * QC + j // 4][hs, ts(j % 4, 128)],
                            qn[n][hs, :], start=True, stop=True)
                    e_sb = epool.tile([128, 1024], BF16, tag="e",
                                      name=f"e{n}_{j}")
                    nc.scalar.activation(e_sb[:], s_ps[:], EXP)
                    rem_j = (2 * QC - c) * NJ - j
                    adaptive = qstate["ns"] // max(rem_j, 1)
                    floor = 700 if c == 0 else 380
                    pump(max(adaptive, floor))
                    # o-accum lags by 2 j-steps: the first o-matmul of a
                    # chunk lands after the previous chunk's norm copies
                    # have freed psO, so the PE never waits on that WAR
                    if j >= 2:
                        for hh in range(2):
                            nc.tensor.matmul(
                                o_ps[0:128, ts(hh, 512)],
                                v_tm[(hh, b * NJ + j - 2)][:],
                                e_hist[j - 2][:, ts(hh, 512)],
                                start=(j - 2 == 0), stop=False)
                    e_hist.append(e_sb)
                for jj in (NJ - 2, NJ - 1):
                    ensure(("t", b * NJ + jj))
                    for hh in range(2):
                        nc.tensor.matmul(
                            o_ps[0:128, ts(hh, 512)],
                            v_tm[(hh, b * NJ + jj)][:],
                            e_hist[jj][:, ts(hh, 512)],
                            start=False, stop=(jj == NJ - 1))
                # sums live at psum partition 0: reciprocal reads PSUM
                # directly; broadcast via tiny PE matmuls against ones1
                r_sb = npool.tile([1, 1024], F32, tag="r", name=f"r{n}")
                nc.vector.reciprocal_approx_fast(r_sb[:], o_ps[0:1, :])
                if c < 2 * QC - 1:
                    # stage out of PSUM so psO frees early for the next chunk
                    oc = npool.tile([64, 1024], F32, tag="oc", name=f"oc{n}")
                    nc.vector.tensor_copy(oc[:], o_ps[64:128, :])
                    osrc = oc[0:64, :]
                else:
                    # last chunk: skip the staging copy, shortening the tail
                    osrc = o_ps[64:128, :]
                for hh in range(2):
                    hs = slice(hh * 64, (hh + 1) * 64)
                    rb = npool.tile([64, 512], F32, tag=f"rb{hh}",
                                    name=f"rb{n}_{hh}")
                    nc.gpsimd.partition_broadcast(
                        rb[:], r_sb[0:1, ts(hh, 512)])
                    nc.vector.tensor_tensor(
                        out=on[n][hs, :], in0=osrc[:, ts(hh, 512)],
                        in1=rb[:], op=mybir.AluOpType.mult)

            # ---- schedule ----
            # head: chunks 0 and 1 projected directly; the rest bridges.
            # Bridge order: k/v/transposes of chunks 2,3 first (batch-0
            # attention needs them early), their q-projections after, then
            # batch-1 chunks in full.
            for n in (0, 1):
                kvt, qu = ph1_units(n)
                for u in kvt + qu:
                    u[1]()
                    if u[2] is not None:
                        done.add(u[2])
            kvt2, qu2 = ph1_units(2)
            kvt3, qu3 = ph1_units(3)
            push(kvt2)
            push(kvt3)
            push(qu2)
            push(qu3)
            for n in range(4, NCHUNK):
                kvt, qu = ph1_units(n)
                push(kvt)
                push(qu)
            # out-projection of chunk c-2 is prepended at chunk c's start:
            # it is guaranteed normalized and fills the chunk-boundary
            # window (o-psum WAR + norm chain) so the PE never goes cold.
            last = 2 * QC - 1
            for c in range(2 * QC):
                pre = []
                if c >= 2:
                    pre += outproj_units(c - 2)
                if c == last:
                    pre += outproj_units(last - 1)[:4]
                for u in reversed(pre):
                    units.appendleft(u)
                    qstate["ns"] += u[0]
                emit_attn(c)
            push(outproj_units(last - 1)[4:])
            push(outproj_units(last))
            pump(10 ** 9)   # drain whatever is left

    nc.compile()
    _CACHE["nc"] = nc
    return nc


def _prep_in_maps(x, Wq, bq, Wk, bk, Wv, Wo):
    import ml_dtypes
    bf16 = ml_dtypes.bfloat16
    xT = np.ascontiguousarray(x.reshape(T, D).T).astype(bf16)
    scale = np.float32(1.0 / np.sqrt(DH))
    in_maps = []
    for c in range(NC):
        sl = slice(128 * c, 128 * (c + 1))
        in_maps.append({
            "xT": xT,
            "wq": np.ascontiguousarray((scale * Wq[sl, :]).T).astype(bf16),
            "wk": np.ascontiguousarray(Wk[sl, :].T).astype(bf16),
            "wv": np.ascontiguousarray(Wv[sl, :].T).astype(bf16),
            "wo": np.ascontiguousarray(Wo[:, sl].T).astype(bf16),
            "bq": np.ascontiguousarray((scale * bq[sl])[:, None]),
            "bk": np.ascontiguousarray(bk[sl][:, None]),
        })
    return in_maps


def kernel(x, Wq, bq, Wk, bk, Wv, bv, Wo, bo):
    x = np.asarray(x, np.float32)
    Wq, bq = np.asarray(Wq, np.float32), np.asarray(bq, np.float32)
    Wk, bk = np.asarray(Wk, np.float32), np.asarray(bk, np.float32)
    Wv, bv = np.asarray(Wv, np.float32), np.asarray(bv, np.float32)
    Wo, bo = np.asarray(Wo, np.float32), np.asarray(bo, np.float32)

    nc = _build()
    in_maps = _prep_in_maps(x, Wq, bq, Wk, bk, Wv, Wo)
    res = bass_utils.run_bass_kernel_spmd(nc, in_maps, core_ids=list(range(NC)))

    acc = np.zeros((D, T), np.float64)
    for c in range(NC):
        acc += res.results[c]["outT"]
    # v-bias folds through softmax (rows sum to 1): + bv @ Wo.T; plus bo.
    const = bo.astype(np.float64) + bv.astype(np.float64) @ Wo.T.astype(np.float64)
    out = acc.T + const[None, :]
    return out.astype(np.float32).reshape(B, S, D)
